# revision 41
# baseline (speedup 1.0000x reference)
"""DilatedRNN Trainium2 Bass kernel, v5: psum-resident pre-activations,
host-side output reassembly.

Math per layer j (dilation d=2**j):
  h_t = tanh(x_t @ Wx_j + h_{t-d} @ Wh_j + b_j),  h_{<0} = 0.

Per core (BL=4 sequences, data-parallel over batch):
  - hr[j]: SBUF ring [128, s, t%512, k] bf16 of layer outputs, transposed
    (partitions = H-rows of chunk k; cols = tokens). xTr: same for x.
  - zw[j]: PSUM rolling windows of step-slots. One accumulation group
    per PSUM bank at a time (hardware resets the bank's accumulation
    state on any start=True): per 64-token bank cycle, one bias matmul
    (1-partition ones outer product, start=True) opens the bank, Wx
    matmuls accumulate in 16-token sub-chunks, the recurrence's Wh
    matmuls accumulate per step (stop=True on each step's last), one
    tanh activation per step reads the slot and writes the ring.
    Layer 0 ping-pongs two banks so its chain never waits on the bulk;
    layers 1-3 (slack chains) use one bank each.
  - Outputs: the bf16 rings are DMA'd straight to DRAM in transposed
    layout; the host untransposes, casts, and applies the ragged mask
    (bitwise identical to on-device masking since outputs already
    round-trip through the bf16 ring).
  - All work units are emitted sorted by virtual token-time so the four
    layers' chains interleave ~LAG tokens apart.
  - Post-compile, _swap_event_sem_waits moves each standalone
    EventSemaphore's cross-engine wait onto the following instruction so
    decode pipelines instead of blocking the sequencer.
"""

import numpy as np

B, T, H, DEPTH = 32, 2048, 256, 4
NCORES = 8
BL = B // NCORES          # sequences per core (4)
NTOK = BL * T             # tokens per core (8192)
P = 128
KC = H // P               # K chunks (2)
MC = H // P               # M chunks (2)

WIN = 512                 # h ring window (tokens per sequence)
CHB = 8                   # bulk sub-chunk (tokens, all seqs at once)
BANK = 64                 # psum bank cycle (tokens)
CHO = 256                 # output DMA block (tokens of one seq)
LAG = 12                  # virtual-time lag per layer (must exceed CHB)

_CACHE = {}


def _swap_event_sem_waits(nc, engines):
    """For each standalone EventSemaphore carrying one cross-engine wait
    whose next same-engine instruction has either no attached wait or a
    trivially-ordered same-engine wait: attach the cross-engine wait to
    the instruction (so it parks in the wait queue with decode already
    done) and leave the trivial wait on the EventSemaphore. Ordering
    constraints are unchanged; only the wait carriers swap."""
    import concourse.mybir as mybir

    fn = nc.m.functions[0]
    upd = {}
    for bb in fn.blocks:
        for inst in bb.instructions:
            si = inst.sync_info
            if not si:
                continue
            for u in (si.on_update or []):
                upd.setdefault(u.id, set()).add(inst.engine)
    nswap = 0
    for bb in fn.blocks:
        prev_es = {}
        for inst in bb.instructions:
            eng = inst.engine
            if eng not in engines:
                continue
            if isinstance(inst, mybir.InstEventSemaphore):
                si = inst.sync_info
                if si and len(si.on_wait or []) == 1 and not (si.on_update or []):
                    w = si.on_wait[0]
                    if eng not in upd.get(w.id, set()):
                        prev_es[eng] = inst
                        continue
                prev_es[eng] = None
                continue
            es = prev_es.pop(eng, None)
            if es is None:
                continue
            si = inst.sync_info
            w_cross = es.sync_info.on_wait[0]
            if si is None or len(si.on_wait or []) == 0:
                es.sync_info = mybir.SyncInfo(on_wait=[], on_update=[])
                inst.sync_info = mybir.SyncInfo(
                    on_wait=[w_cross],
                    on_update=list(si.on_update or []) if si else [])
                nswap += 1
                continue
            if len(si.on_wait) != 1:
                continue
            w_self = si.on_wait[0]
            if upd.get(w_self.id, set()) != {eng}:
                continue
            es.sync_info = mybir.SyncInfo(on_wait=[w_self], on_update=[])
            inst.sync_info = mybir.SyncInfo(
                on_wait=[w_cross], on_update=list(si.on_update or []))
            nswap += 1
    return nswap


def _build_program(TE=T):
    # TE: effective token count (multiple of BANK, <= T). Tokens beyond
    # TE are masked out for every sequence and never computed; the host
    # zero-fills them.
    import concourse.bacc as bacc
    import concourse.mybir as mybir
    import concourse.tile as tile

    fp32 = mybir.dt.float32
    bf16 = mybir.dt.bfloat16

    nc = bacc.Bacc("TRN2", target_bir_lowering=False, debug=False,
                   num_devices=NCORES)

    x_in = nc.dram_tensor("x", [NTOK, H], fp32, kind="ExternalInput")
    w_in = nc.dram_tensor("w", [P, DEPTH * 2 * KC * MC * P], bf16,
                          kind="ExternalInput")
    b_in = nc.dram_tensor("b", [MC, DEPTH * P], bf16,
                          kind="ExternalInput")
    ind_in = nc.dram_tensor("indic", [MC, DEPTH * BANK * MC * BL], bf16,
                            kind="ExternalInput")
    ident_in = nc.dram_tensor("ident", [P, P], fp32, kind="ExternalInput")
    # transposed output: value (j, s, p, t, k) = h_j[seq s, token t,
    # H-row k*128+p], flattened as [DEPTH, BL, P, T*KC] bf16.
    out_t = nc.dram_tensor("out", [DEPTH, BL, P, T * KC], bf16,
                           kind="ExternalOutput")

    with tile.TileContext(nc) as tc:
        with (
            tc.tile_pool(name="const", bufs=1) as constp,
            tc.tile_pool(name="rings", bufs=1) as ringp,
            tc.tile_pool(name="xload", bufs=4) as xloadp,
            tc.tile_pool(name="zw", bufs=1, space="PSUM") as zwp,
            tc.tile_pool(name="ps_tr", bufs=2, space="PSUM") as ps_tr,
        ):
            WL = 2 * KC * MC * P  # weight cols per layer (1024)
            wsb = constp.tile([P, DEPTH * WL], bf16, name="wsb")
            bsb = constp.tile([MC, DEPTH * P], bf16, name="bsb")
            nc.sync.dma_start(bsb[:], b_in[:])
            idsb = constp.tile([P, P], fp32, name="idsb")
            nc.sync.dma_start(idsb[:], ident_in[:])
            idbf = constp.tile([P, P], bf16, name="idbf")
            nc.vector.tensor_copy(idbf[:], idsb[:])
            # per-layer m-chunk indicator: ind[j][i, (q, m, s*r)] = (i == m)
            LW = BANK * MC * BL  # 512 cols per layer
            indsb = constp.tile([MC, DEPTH * LW], bf16, name="indsb")
            nc.sync.dma_start(indsb[:], ind_in[:])
            ind = [indsb[:, j * LW:(j + 1) * LW] for j in range(DEPTH)]
            zt = constp.tile([P, BL * 8], bf16, name="zt")
            nc.vector.memset(zt[:], 0.0)
            ztv = zt.rearrange("p (s r) -> p s r", s=BL)
            # dummy act: absorb the Tanh table load before the chain starts
            warm = constp.tile([P, 1], fp32, name="warm")
            nc.scalar.activation(warm[:], zt[:, 0:1],
                                 mybir.ActivationFunctionType.Tanh)

            def wslice(j, mat, k, m):
                col = (((j * 2 + mat) * KC + k) * MC + m) * P
                return wsb[:, col:col + P]

            # x ring, same layout as h rings: [p, s, t%WIN, k]
            xTr = ringp.tile([P, BL * WIN * KC], bf16, name="xTr", tag="xTr")
            xTrv = xTr.rearrange("p (s t k) -> p s t k", s=BL, k=KC)

            hr, hrv = [], []
            # psum bank tiles: layer 0 gets two (ping-pong), others one.
            # each holds BANK tokens of slots: BANK/d slots of W2 cols.
            zbank = {0: [], 1: [], 2: [], 3: []}
            zbv = {0: [], 1: [], 2: [], 3: []}
            for j in range(DEPTH):
                d = 1 << j
                W2 = MC * BL * d
                h_t = ringp.tile([P, BL * WIN * KC], bf16, name=f"hr{j}",
                                 tag=f"hr{j}")
                hr.append(h_t)
                hrv.append(h_t.rearrange("p (s t k) -> p s t k", s=BL, k=KC))
                nb = 2 if j == 0 else 1
                for i in range(nb):
                    z_t = zwp.tile([P, (BANK // d) * W2], fp32,
                                   name=f"zw{j}_{i}", tag=f"zw{j}_{i}")
                    zbank[j].append(z_t)
                    zbv[j].append(z_t.rearrange("p (n w) -> p n w", w=W2))

            events = []  # (v, tie, seq, fn)

            def add(v, tie, fn):
                events.append((v, tie, len(events), fn))

            # ---- x stage: per (seq, 128-token block): load + transpose ----
            def mk_xstage(s_seq, tb):
                def fn():
                    fl = s_seq * T + tb
                    xnat = xloadp.tile([P, H], fp32, name="xnat", tag="xn")
                    nc.sync.dma_start(xnat[:], x_in[fl:fl + P, :])
                    xbf = xloadp.tile([P, H], bf16, name="xbf", tag="xb")
                    nc.vector.tensor_copy(xbf[:], xnat[:])
                    ro = tb % WIN
                    for k in range(KC):
                        # bf16 transpose: 1 cyc/row keeps the PE burst
                        # short (f32 would HOL-block the chain's matmuls)
                        xtp = ps_tr.tile([P, P], bf16, name="xtp", tag="tr")
                        nc.tensor.transpose(xtp[:],
                                            xbf[:, k * P:(k + 1) * P],
                                            idbf[:])
                        nc.vector.tensor_copy(xTrv[:, s_seq, ro:ro + P, k],
                                              xtp[:])
                return fn

            for tb in range(0, TE, P):
                for s_seq in range(BL):
                    add(tb - 400.0, 0, mk_xstage(s_seq, tb))

            # weights dispatched after x block 0 so the x pipeline (which
            # feeds the chain's first links) isn't stuck behind the big
            # weight transfer on the DMA dispatch queue.
            add(-399.8, 0, lambda: nc.sync.dma_start(
                wsb[:, 0:WL], w_in[:, 0:WL]))
            add(-399.5, 0, lambda: nc.sync.dma_start(
                wsb[:, WL:], w_in[:, WL:]))

            # ---- bias: open each bank cycle with ONE start=True matmul
            # covering the whole bank: out[p, c] = sum_i b[j][i*128+p]
            # * ind[i, c], contraction over MC=2 partitions. ----
            # split into 128-col pieces: a single 512-col matmul (up to
            # ~790ns at cold PE clock) would HOL-block the chain's Wh
            # matmuls in the in-order PE queue. Only the first piece
            # carries start=True (resets the bank); later pieces land on
            # reset (unwritten) regions where start=False writes cleanly.
            def mk_bias_piece(j, c0, p0):
                nb = len(zbank[j])
                def fn():
                    z_t = zbank[j][(c0 // BANK) % nb]
                    nc.tensor.matmul(z_t[:, p0:p0 + P],
                                     bsb[:, j * P:(j + 1) * P],
                                     ind[j][:, p0:p0 + P],
                                     start=(p0 == 0), stop=False)
                return fn

            # ---- bulk: accumulate x@Wx into psum slots, CHB tokens.
            # emitted per m-chunk at spread v so the bursts interleave
            # with chain matmuls instead of HOL-blocking them. ----
            def mk_bulk(j, t0, m):
                d = 1 << j
                bd = BL * d
                nb = len(zbank[j])
                nq = CHB // d
                def fn():
                    rv = xTrv if j == 0 else hrv[j - 1]
                    zv = zbv[j][(t0 // BANK) % nb]
                    q0 = (t0 % BANK) // d
                    out3 = zv[:, q0:q0 + nq,
                              m * bd:(m + 1) * bd].rearrange(
                        "p q (s r) -> p q s r", s=BL)
                    for k in range(KC):
                        rhs3 = rv[:, :, t0 % WIN: t0 % WIN + CHB,
                                  k].rearrange(
                            "p s (q r) -> p q s r", r=d)
                        nc.tensor.matmul(out3, wslice(j, 0, k, m), rhs3,
                                         start=False, stop=False)
                return fn

            for t0 in range(0, TE, CHB):
                # layer 0: fully off-chain (double-banked)
                if t0 % BANK == 0:
                    # all pieces before the first bulk, spread 2 links
                    for p in range(4):
                        add(t0 - 44.0 + 2.0 * p, 2,
                            mk_bias_piece(0, t0, P * p))
                add(t0 - 32.0, 2, mk_bulk(0, t0, 0))
                add(t0 - 29.0, 2, mk_bulk(0, t0, 1))
            for j in range(1, DEPTH):
                d = 1 << j
                m1off = min(3.0, d - 0.5)
                for t0 in range(0, TE, CHB):
                    c0 = (t0 // BANK) * BANK
                    if t0 % BANK == 0:
                        # single bank: reopen only after the previous
                        # cycle's last act (v = c0 + j*LAG, tie 1);
                        # pieces spread 2 links apart
                        for p in range(4):
                            add(c0 + j * LAG + 0.02 + 2.0 * p, 2,
                                mk_bias_piece(j, t0, P * p))
                    v = max(t0 + CHB + (j - 1) * LAG, c0 + j * LAG + 0.05)
                    add(v, 2, mk_bulk(j, t0, 0))
                    add(v + m1off, 2, mk_bulk(j, t0, 1))

            # ---- recurrence step: Wh accumulate + tanh. For L2/L3 the
            # two m-groups are emitted at spread v so their bursts don't
            # HOL-block layer 0's chain matmuls. ----
            def mk_step_mm(j, n, ms):
                d = 1 << j
                bd = BL * d
                nb = len(zbank[j])
                spb = BANK // d  # slots per bank
                def fn():
                    zv = zbv[j][(n // spb) % nb]
                    zslot = zv[:, n % spb, :]
                    for m in ms:
                        for k in range(KC):
                            if n > 0:
                                ro = ((n - 1) * d) % WIN
                                rhs = hrv[j][:, :, ro:ro + d, k]
                            else:
                                rhs = ztv[:, :, 0:d]
                            nc.tensor.matmul(
                                zslot[:, m * bd:(m + 1) * bd],
                                wslice(j, 1, k, m), rhs,
                                start=False,
                                stop=(m == MC - 1 and k == KC - 1))
                return fn

            def mk_step_act(j, n):
                d = 1 << j
                bd = BL * d
                nb = len(zbank[j])
                spb = BANK // d
                def fn():
                    zv = zbv[j][(n // spb) % nb]
                    zslot = zv[:, n % spb, :]
                    wo = (n * d) % WIN
                    dst = hrv[j][:, :, wo:wo + d, :].rearrange(
                        "p s r k -> p k s r")
                    nc.scalar.activation(dst, zslot,
                                         mybir.ActivationFunctionType.Tanh)
                return fn

            for j in range(DEPTH):
                d = 1 << j
                for n in range(TE // d):
                    v = float((n + 1) * d + j * LAG)
                    add(v, 1, mk_step_mm(j, n, range(MC)))
                    add(v, 1, mk_step_act(j, n))

            # ---- output: DMA the ring straight to DRAM (transposed) ----
            def mk_out(j, s_seq, tb):
                def fn():
                    ro = tb % WIN
                    src = hrv[j][:, s_seq, ro:ro + CHO, :]
                    nc.sync.dma_start(
                        out_t[j, s_seq, :, tb * KC:(tb + CHO) * KC], src)
                return fn

            for j in range(DEPTH):
                for tb in range(0, TE, CHO):
                    for s_seq in range(BL):
                        add(tb + CHO + j * LAG + 0.5, 3,
                            mk_out(j, s_seq, tb))

            events.sort(key=lambda e: (e[0], e[1], e[2]))
            for _, _, _, fn in events:
                fn()

    nc.compile()
    import concourse.mybir as _mybir
    _swap_event_sem_waits(nc, (
        _mybir.EngineType.Activation, _mybir.EngineType.PE,
        _mybir.EngineType.DVE))
    return nc


def _get_program(TE=T):
    key = ("nc", TE)
    if key not in _CACHE:
        _CACHE[key] = _build_program(TE)
    return _CACHE[key]


def _prepare_in_maps(x, Wx, Wh, b, lens):
    import ml_dtypes

    bf = ml_dtypes.bfloat16
    wbig = np.empty((P, DEPTH * 2 * KC * MC * P), dtype=bf)
    for j in range(DEPTH):
        for mat, Wm in ((0, Wx), (1, Wh)):
            for k in range(KC):
                for m in range(MC):
                    col = (((j * 2 + mat) * KC + k) * MC + m) * P
                    wbig[:, col:col + P] = Wm[j][k * P:(k + 1) * P,
                                                 m * P:(m + 1) * P].astype(bf)
    bbig = np.empty((MC, DEPTH * P), dtype=bf)
    for j in range(DEPTH):
        for m in range(MC):
            bbig[m, j * P:(j + 1) * P] = b[j][m * P:(m + 1) * P].astype(bf)
    ident = np.eye(P, dtype=np.float32)
    # indicator: per layer j, cols (q, m, s*r): row i == m
    LW = BANK * MC * BL
    indic = np.zeros((MC, DEPTH * LW), dtype=bf)
    for j in range(DEPTH):
        d = 1 << j
        bd = BL * d
        pat = np.zeros((MC, BANK // d, MC, bd), dtype=bf)
        for m in range(MC):
            pat[m, :, m, :] = 1.0
        indic[:, j * LW:(j + 1) * LW] = pat.reshape(MC, LW)

    in_maps = []
    for c in range(NCORES):
        xs = np.ascontiguousarray(
            x[c * BL:(c + 1) * BL].reshape(NTOK, H).astype(np.float32))
        in_maps.append({
            "x": xs, "w": wbig, "b": bbig, "ident": ident, "indic": indic,
        })
    return in_maps


def kernel(x, Wx, Wh, b, seq_lens):
    from concourse import bass_utils

    x = np.asarray(x)
    Wx = np.asarray(Wx)
    Wh = np.asarray(Wh)
    b = np.asarray(b)
    lens = np.asarray(seq_lens).astype(np.int64)

    in_maps = _prepare_in_maps(x, Wx, Wh, b, lens)

    # tokens past the longest sequence are masked to zero for every batch
    # element; skip computing them (host zero-fills).
    max_len = int(lens.max())
    TE = min(T, ((max_len + BANK - 1) // BANK) * BANK)
    nc = _get_program(TE)
    res = bass_utils.run_bass_kernel_spmd(
        nc, in_maps, core_ids=list(range(NCORES)), trace=False)
    _CACHE["last_result"] = res

    out = np.zeros((B, DEPTH, T, H), dtype=np.float32)
    for c in range(NCORES):
        raw = np.asarray(res.results[c]["out"])  # [D, BL, P, T*KC] bf16
        raw = raw.reshape(DEPTH, BL, P, T, KC).astype(np.float32)
        # (j, s, p, t, k) -> (s, j, t, k, p); H index = k*128 + p
        oc = raw.transpose(1, 0, 3, 4, 2).reshape(BL, DEPTH, T, H)
        oc[:, :, TE:, :] = 0.0
        mask = (np.arange(T)[None, :] < lens[c * BL:(c + 1) * BL][:, None])
        out[c * BL:(c + 1) * BL] = oc * mask[:, None, :, None].astype(
            np.float32)
    return out


# revision 43
# speedup vs baseline: 1.0002x; 1.0002x over previous
"""DilatedRNN Trainium2 Bass kernel, v5: psum-resident pre-activations,
host-side output reassembly.

Math per layer j (dilation d=2**j):
  h_t = tanh(x_t @ Wx_j + h_{t-d} @ Wh_j + b_j),  h_{<0} = 0.

Per core (BL=4 sequences, data-parallel over batch):
  - hr[j]: SBUF ring [128, s, t%512, k] bf16 of layer outputs, transposed
    (partitions = H-rows of chunk k; cols = tokens). xTr: same for x.
  - zw[j]: PSUM rolling windows of step-slots. One accumulation group
    per PSUM bank at a time (hardware resets the bank's accumulation
    state on any start=True): per 64-token bank cycle, one bias matmul
    (1-partition ones outer product, start=True) opens the bank, Wx
    matmuls accumulate in 16-token sub-chunks, the recurrence's Wh
    matmuls accumulate per step (stop=True on each step's last), one
    tanh activation per step reads the slot and writes the ring.
    Layer 0 ping-pongs two banks so its chain never waits on the bulk;
    layers 1-3 (slack chains) use one bank each.
  - Outputs: the bf16 rings are DMA'd straight to DRAM in transposed
    layout; the host untransposes, casts, and applies the ragged mask
    (bitwise identical to on-device masking since outputs already
    round-trip through the bf16 ring).
  - All work units are emitted sorted by virtual token-time so the four
    layers' chains interleave ~LAG tokens apart.
  - Post-compile, _swap_event_sem_waits moves each standalone
    EventSemaphore's cross-engine wait onto the following instruction so
    decode pipelines instead of blocking the sequencer.
"""

import numpy as np

B, T, H, DEPTH = 32, 2048, 256, 4
NCORES = 8
BL = B // NCORES          # sequences per core (4)
NTOK = BL * T             # tokens per core (8192)
P = 128
KC = H // P               # K chunks (2)
MC = H // P               # M chunks (2)

WIN = 512                 # h ring window (tokens per sequence)
CHB = 8                   # bulk sub-chunk (tokens, all seqs at once)
BANK = 64                 # psum bank cycle (tokens)
CHO = 256                 # output DMA block (tokens of one seq)
LAG = 12                  # virtual-time lag per layer (must exceed CHB)

_CACHE = {}


def _swap_event_sem_waits(nc, engines):
    """For each standalone EventSemaphore carrying one cross-engine wait
    whose next same-engine instruction has either no attached wait or a
    trivially-ordered same-engine wait: attach the cross-engine wait to
    the instruction (so it parks in the wait queue with decode already
    done) and leave the trivial wait on the EventSemaphore. Ordering
    constraints are unchanged; only the wait carriers swap."""
    import concourse.mybir as mybir

    fn = nc.m.functions[0]
    upd = {}
    for bb in fn.blocks:
        for inst in bb.instructions:
            si = inst.sync_info
            if not si:
                continue
            for u in (si.on_update or []):
                upd.setdefault(u.id, set()).add(inst.engine)
    nswap = 0
    for bb in fn.blocks:
        prev_es = {}
        for inst in bb.instructions:
            eng = inst.engine
            if eng not in engines:
                continue
            if isinstance(inst, mybir.InstEventSemaphore):
                si = inst.sync_info
                if si and len(si.on_wait or []) == 1 and not (si.on_update or []):
                    w = si.on_wait[0]
                    if eng not in upd.get(w.id, set()):
                        prev_es[eng] = inst
                        continue
                prev_es[eng] = None
                continue
            es = prev_es.pop(eng, None)
            if es is None:
                continue
            si = inst.sync_info
            w_cross = es.sync_info.on_wait[0]
            if si is None or len(si.on_wait or []) == 0:
                es.sync_info = mybir.SyncInfo(on_wait=[], on_update=[])
                inst.sync_info = mybir.SyncInfo(
                    on_wait=[w_cross],
                    on_update=list(si.on_update or []) if si else [])
                nswap += 1
                continue
            if len(si.on_wait) != 1:
                continue
            w_self = si.on_wait[0]
            if upd.get(w_self.id, set()) != {eng}:
                continue
            es.sync_info = mybir.SyncInfo(on_wait=[w_self], on_update=[])
            inst.sync_info = mybir.SyncInfo(
                on_wait=[w_cross], on_update=list(si.on_update or []))
            nswap += 1
    return nswap


def _build_program(TE=T):
    # TE: effective token count (multiple of BANK, <= T). Tokens beyond
    # TE are masked out for every sequence and never computed; the host
    # zero-fills them.
    import concourse.bacc as bacc
    import concourse.mybir as mybir
    import concourse.tile as tile

    fp32 = mybir.dt.float32
    bf16 = mybir.dt.bfloat16

    nc = bacc.Bacc("TRN2", target_bir_lowering=False, debug=False,
                   num_devices=NCORES)

    x_in = nc.dram_tensor("x", [NTOK, H], fp32, kind="ExternalInput")
    w_in = nc.dram_tensor("w", [P, DEPTH * 2 * KC * MC * P], bf16,
                          kind="ExternalInput")
    b_in = nc.dram_tensor("b", [MC, DEPTH * P], bf16,
                          kind="ExternalInput")
    ind_in = nc.dram_tensor("indic", [MC, DEPTH * BANK * MC * BL], bf16,
                            kind="ExternalInput")
    ident_in = nc.dram_tensor("ident", [P, P], fp32, kind="ExternalInput")
    # transposed output: value (j, s, p, t, k) = h_j[seq s, token t,
    # H-row k*128+p], flattened as [DEPTH, BL, P, T*KC] bf16.
    out_t = nc.dram_tensor("out", [DEPTH, BL, P, T * KC], bf16,
                           kind="ExternalOutput")

    with tile.TileContext(nc) as tc:
        with (
            tc.tile_pool(name="const", bufs=1) as constp,
            tc.tile_pool(name="rings", bufs=1) as ringp,
            tc.tile_pool(name="xload", bufs=4) as xloadp,
            tc.tile_pool(name="zw", bufs=1, space="PSUM") as zwp,
            tc.tile_pool(name="ps_tr", bufs=2, space="PSUM") as ps_tr,
        ):
            WL = 2 * KC * MC * P  # weight cols per layer (1024)
            wsb = constp.tile([P, DEPTH * WL], bf16, name="wsb")
            bsb = constp.tile([MC, DEPTH * P], bf16, name="bsb")
            nc.sync.dma_start(bsb[:], b_in[:])
            idsb = constp.tile([P, P], fp32, name="idsb")
            nc.sync.dma_start(idsb[:], ident_in[:])
            idbf = constp.tile([P, P], bf16, name="idbf")
            nc.vector.tensor_copy(idbf[:], idsb[:])
            # per-layer m-chunk indicator: ind[j][i, (q, m, s*r)] = (i == m)
            LW = BANK * MC * BL  # 512 cols per layer
            indsb = constp.tile([MC, DEPTH * LW], bf16, name="indsb")
            nc.sync.dma_start(indsb[:], ind_in[:])
            ind = [indsb[:, j * LW:(j + 1) * LW] for j in range(DEPTH)]
            zt = constp.tile([P, BL * 8], bf16, name="zt")
            nc.vector.memset(zt[:], 0.0)
            ztv = zt.rearrange("p (s r) -> p s r", s=BL)
            # dummy act: absorb the Tanh table load before the chain starts
            warm = constp.tile([P, 1], fp32, name="warm")
            nc.scalar.activation(warm[:], zt[:, 0:1],
                                 mybir.ActivationFunctionType.Tanh)

            def wslice(j, mat, k, m):
                col = (((j * 2 + mat) * KC + k) * MC + m) * P
                return wsb[:, col:col + P]

            # x ring, same layout as h rings: [p, s, t%WIN, k]
            xTr = ringp.tile([P, BL * WIN * KC], bf16, name="xTr", tag="xTr")
            xTrv = xTr.rearrange("p (s t k) -> p s t k", s=BL, k=KC)

            hr, hrv = [], []
            # psum bank tiles: layer 0 gets two (ping-pong), others one.
            # each holds BANK tokens of slots: BANK/d slots of W2 cols.
            zbank = {0: [], 1: [], 2: [], 3: []}
            zbv = {0: [], 1: [], 2: [], 3: []}
            for j in range(DEPTH):
                d = 1 << j
                W2 = MC * BL * d
                h_t = ringp.tile([P, BL * WIN * KC], bf16, name=f"hr{j}",
                                 tag=f"hr{j}")
                hr.append(h_t)
                hrv.append(h_t.rearrange("p (s t k) -> p s t k", s=BL, k=KC))
                nb = 2 if j == 0 else 1
                for i in range(nb):
                    z_t = zwp.tile([P, (BANK // d) * W2], fp32,
                                   name=f"zw{j}_{i}", tag=f"zw{j}_{i}")
                    zbank[j].append(z_t)
                    zbv[j].append(z_t.rearrange("p (n w) -> p n w", w=W2))

            events = []  # (v, tie, seq, fn)

            def add(v, tie, fn):
                events.append((v, tie, len(events), fn))

            # ---- x stage: per (seq, 128-token block): load + transpose ----
            def mk_xstage(s_seq, tb):
                def fn():
                    fl = s_seq * T + tb
                    xnat = xloadp.tile([P, H], fp32, name="xnat", tag="xn")
                    nc.sync.dma_start(xnat[:], x_in[fl:fl + P, :])
                    xbf = xloadp.tile([P, H], bf16, name="xbf", tag="xb")
                    nc.vector.tensor_copy(xbf[:], xnat[:])
                    ro = tb % WIN
                    for k in range(KC):
                        # bf16 transpose: 1 cyc/row keeps the PE burst
                        # short (f32 would HOL-block the chain's matmuls)
                        xtp = ps_tr.tile([P, P], bf16, name="xtp", tag="tr")
                        nc.tensor.transpose(xtp[:],
                                            xbf[:, k * P:(k + 1) * P],
                                            idbf[:])
                        nc.vector.tensor_copy(xTrv[:, s_seq, ro:ro + P, k],
                                              xtp[:])
                return fn

            for tb in range(0, TE, P):
                for s_seq in range(BL):
                    add(tb - 400.0, 0, mk_xstage(s_seq, tb))

            # weights dispatched after x block 0 so the x pipeline (which
            # feeds the chain's first links) isn't stuck behind the big
            # weight transfer on the DMA dispatch queue. Wx0 (first half)
            # is its own transfer so the first bulk doesn't wait for Wh0.
            def load_w0():
                nc.sync.dma_start(wsb[:, 0:WL // 2], w_in[:, 0:WL // 2])
                nc.sync.dma_start(wsb[:, WL // 2:WL], w_in[:, WL // 2:WL])

            add(-399.8, 0, load_w0)
            add(-399.5, 0, lambda: nc.sync.dma_start(
                wsb[:, WL:], w_in[:, WL:]))

            # ---- bias: open each bank cycle with ONE start=True matmul
            # covering the whole bank: out[p, c] = sum_i b[j][i*128+p]
            # * ind[i, c], contraction over MC=2 partitions. ----
            # split into 128-col pieces: a single 512-col matmul (up to
            # ~790ns at cold PE clock) would HOL-block the chain's Wh
            # matmuls in the in-order PE queue. Only the first piece
            # carries start=True (resets the bank); later pieces land on
            # reset (unwritten) regions where start=False writes cleanly.
            def mk_bias_piece(j, c0, p0):
                nb = len(zbank[j])
                def fn():
                    z_t = zbank[j][(c0 // BANK) % nb]
                    nc.tensor.matmul(z_t[:, p0:p0 + P],
                                     bsb[:, j * P:(j + 1) * P],
                                     ind[j][:, p0:p0 + P],
                                     start=(p0 == 0), stop=False)
                return fn

            # ---- bulk: accumulate x@Wx into psum slots, CHB tokens.
            # emitted per m-chunk at spread v so the bursts interleave
            # with chain matmuls instead of HOL-blocking them. ----
            def mk_bulk(j, t0, m):
                d = 1 << j
                bd = BL * d
                nb = len(zbank[j])
                nq = CHB // d
                def fn():
                    rv = xTrv if j == 0 else hrv[j - 1]
                    zv = zbv[j][(t0 // BANK) % nb]
                    q0 = (t0 % BANK) // d
                    out3 = zv[:, q0:q0 + nq,
                              m * bd:(m + 1) * bd].rearrange(
                        "p q (s r) -> p q s r", s=BL)
                    for k in range(KC):
                        rhs3 = rv[:, :, t0 % WIN: t0 % WIN + CHB,
                                  k].rearrange(
                            "p s (q r) -> p q s r", r=d)
                        nc.tensor.matmul(out3, wslice(j, 0, k, m), rhs3,
                                         start=False, stop=False)
                return fn

            for t0 in range(0, TE, CHB):
                # layer 0: fully off-chain (double-banked)
                if t0 % BANK == 0:
                    # all pieces before the first bulk, spread 2 links
                    for p in range(4):
                        add(t0 - 44.0 + 2.0 * p, 2,
                            mk_bias_piece(0, t0, P * p))
                add(t0 - 32.0, 2, mk_bulk(0, t0, 0))
                add(t0 - 29.0, 2, mk_bulk(0, t0, 1))
            for j in range(1, DEPTH):
                d = 1 << j
                m1off = min(3.0, d - 0.5)
                for t0 in range(0, TE, CHB):
                    c0 = (t0 // BANK) * BANK
                    if t0 % BANK == 0:
                        # single bank: reopen only after the previous
                        # cycle's last act (v = c0 + j*LAG, tie 1);
                        # pieces spread 2 links apart
                        for p in range(4):
                            add(c0 + j * LAG + 0.02 + 2.0 * p, 2,
                                mk_bias_piece(j, t0, P * p))
                    v = max(t0 + CHB + (j - 1) * LAG, c0 + j * LAG + 0.05)
                    add(v, 2, mk_bulk(j, t0, 0))
                    add(v + m1off, 2, mk_bulk(j, t0, 1))

            # ---- recurrence step: Wh accumulate + tanh. For L2/L3 the
            # two m-groups are emitted at spread v so their bursts don't
            # HOL-block layer 0's chain matmuls. ----
            def mk_step_mm(j, n, ms):
                d = 1 << j
                bd = BL * d
                nb = len(zbank[j])
                spb = BANK // d  # slots per bank
                def fn():
                    zv = zbv[j][(n // spb) % nb]
                    zslot = zv[:, n % spb, :]
                    for m in ms:
                        for k in range(KC):
                            if n > 0:
                                ro = ((n - 1) * d) % WIN
                                rhs = hrv[j][:, :, ro:ro + d, k]
                            else:
                                rhs = ztv[:, :, 0:d]
                            nc.tensor.matmul(
                                zslot[:, m * bd:(m + 1) * bd],
                                wslice(j, 1, k, m), rhs,
                                start=False,
                                stop=(m == MC - 1 and k == KC - 1))
                return fn

            def mk_step_act(j, n):
                d = 1 << j
                bd = BL * d
                nb = len(zbank[j])
                spb = BANK // d
                def fn():
                    zv = zbv[j][(n // spb) % nb]
                    zslot = zv[:, n % spb, :]
                    wo = (n * d) % WIN
                    dst = hrv[j][:, :, wo:wo + d, :].rearrange(
                        "p s r k -> p k s r")
                    nc.scalar.activation(dst, zslot,
                                         mybir.ActivationFunctionType.Tanh)
                return fn

            for j in range(DEPTH):
                d = 1 << j
                for n in range(TE // d):
                    v = float((n + 1) * d + j * LAG)
                    add(v, 1, mk_step_mm(j, n, range(MC)))
                    add(v, 1, mk_step_act(j, n))

            # ---- output: DMA the ring straight to DRAM (transposed).
            # The final block per (j, seq) is split in half so the first
            # half drains before the chains finish (shorter tail). ----
            def mk_out(j, s_seq, tb, length):
                def fn():
                    ro = tb % WIN
                    src = hrv[j][:, s_seq, ro:ro + length, :]
                    nc.sync.dma_start(
                        out_t[j, s_seq, :, tb * KC:(tb + length) * KC], src)
                return fn

            for j in range(DEPTH):
                for tb in range(0, TE, CHO):
                    for s_seq in range(BL):
                        if tb + CHO >= TE:
                            half = CHO // 2
                            add(tb + half + j * LAG + 0.5, 3,
                                mk_out(j, s_seq, tb, half))
                            add(tb + CHO + j * LAG + 0.5, 3,
                                mk_out(j, s_seq, tb + half, half))
                        else:
                            add(tb + CHO + j * LAG + 0.5, 3,
                                mk_out(j, s_seq, tb, CHO))

            events.sort(key=lambda e: (e[0], e[1], e[2]))
            for _, _, _, fn in events:
                fn()

    nc.compile()
    import concourse.mybir as _mybir
    _swap_event_sem_waits(nc, (
        _mybir.EngineType.Activation, _mybir.EngineType.PE,
        _mybir.EngineType.DVE))
    return nc


def _get_program(TE=T):
    key = ("nc", TE)
    if key not in _CACHE:
        _CACHE[key] = _build_program(TE)
    return _CACHE[key]


def _prepare_in_maps(x, Wx, Wh, b, lens):
    import ml_dtypes

    bf = ml_dtypes.bfloat16
    wbig = np.empty((P, DEPTH * 2 * KC * MC * P), dtype=bf)
    for j in range(DEPTH):
        for mat, Wm in ((0, Wx), (1, Wh)):
            for k in range(KC):
                for m in range(MC):
                    col = (((j * 2 + mat) * KC + k) * MC + m) * P
                    wbig[:, col:col + P] = Wm[j][k * P:(k + 1) * P,
                                                 m * P:(m + 1) * P].astype(bf)
    bbig = np.empty((MC, DEPTH * P), dtype=bf)
    for j in range(DEPTH):
        for m in range(MC):
            bbig[m, j * P:(j + 1) * P] = b[j][m * P:(m + 1) * P].astype(bf)
    ident = np.eye(P, dtype=np.float32)
    # indicator: per layer j, cols (q, m, s*r): row i == m
    LW = BANK * MC * BL
    indic = np.zeros((MC, DEPTH * LW), dtype=bf)
    for j in range(DEPTH):
        d = 1 << j
        bd = BL * d
        pat = np.zeros((MC, BANK // d, MC, bd), dtype=bf)
        for m in range(MC):
            pat[m, :, m, :] = 1.0
        indic[:, j * LW:(j + 1) * LW] = pat.reshape(MC, LW)

    in_maps = []
    for c in range(NCORES):
        xs = np.ascontiguousarray(
            x[c * BL:(c + 1) * BL].reshape(NTOK, H).astype(np.float32))
        in_maps.append({
            "x": xs, "w": wbig, "b": bbig, "ident": ident, "indic": indic,
        })
    return in_maps


def kernel(x, Wx, Wh, b, seq_lens):
    from concourse import bass_utils

    x = np.asarray(x)
    Wx = np.asarray(Wx)
    Wh = np.asarray(Wh)
    b = np.asarray(b)
    lens = np.asarray(seq_lens).astype(np.int64)

    in_maps = _prepare_in_maps(x, Wx, Wh, b, lens)

    # tokens past the longest sequence are masked to zero for every batch
    # element; skip computing them (host zero-fills).
    max_len = int(lens.max())
    TE = min(T, ((max_len + BANK - 1) // BANK) * BANK)
    nc = _get_program(TE)
    res = bass_utils.run_bass_kernel_spmd(
        nc, in_maps, core_ids=list(range(NCORES)), trace=False)
    _CACHE["last_result"] = res

    out = np.zeros((B, DEPTH, T, H), dtype=np.float32)
    for c in range(NCORES):
        raw = np.asarray(res.results[c]["out"])  # [D, BL, P, T*KC] bf16
        raw = raw.reshape(DEPTH, BL, P, T, KC).astype(np.float32)
        # (j, s, p, t, k) -> (s, j, t, k, p); H index = k*128 + p
        oc = raw.transpose(1, 0, 3, 4, 2).reshape(BL, DEPTH, T, H)
        oc[:, :, TE:, :] = 0.0
        mask = (np.arange(T)[None, :] < lens[c * BL:(c + 1) * BL][:, None])
        out[c * BL:(c + 1) * BL] = oc * mask[:, None, :, None].astype(
            np.float32)
    return out
